# revision 1
# baseline (speedup 1.0000x reference)
"""CapsuleNet forward kernel for 8 Trainium2 NeuronCores.

Data-parallel over batch (64 images / core); the routing b_ij batch-mean
uses an AllReduce per iteration.  u_hat is never materialized: s_j and the
agreement mean are computed directly against W from the 9216-dim flattened
capsule vector u.

Per-core pipeline:
  conv1  : one K=81 matmul per output tile (im2col built by a single
           strided DMA from DRAM, 2240B segments, 8 garbage cols/row that
           are cropped during the ReLU copy; conv1 bias folded into the
           ReLU's bias operand)
  conv2  : 324 accumulating K=128 matmuls (81 taps x 2 ci chunks) per co
           chunk over the full local batch (5 image-aligned PSUM banks)
  capsule: scatter-transpose conv2 output to u2T[b, f] (f = co*36+s),
           squash over 8-elem groups, PE-transpose to u2R[f, b]
  routing: s_j^T = (c-scaled W)^T @ u2, 72 K-tile accumulation;
           agreement mean m = sum_{o,i} W .* (v2^T @ u2) via rank-64
           matmul + DVE mult/group-reduce + selector matmuls;
           AllReduce(m) -> b_ij update -> softmax.
"""

import numpy as np
import ml_dtypes

import concourse.bacc as bacc
import concourse.bass as bass
import concourse.mybir as mybir
import concourse.tile as tile
from concourse.bass_utils import run_bass_kernel_spmd

F32 = mybir.dt.float32
BF16 = mybir.dt.bfloat16
MUL = mybir.AluOpType.mult
ADD = mybir.AluOpType.add
MAX = mybir.AluOpType.max
AXX = mybir.AxisListType.X
ACT = mybir.ActivationFunctionType

NCORES = 8
B = 512
BL = B // NCORES        # 64 images per core
SB = 16                 # conv1 im2col sub-batch
NSB = BL // SB
J = 560                 # 20 rows x 28 cols (8 garbage cols/row)
JC = 400                # compact 20x20 conv1 output per image
R, C, O, I = 1152, 10, 16, 8
F = R * I               # 9216
CO = C * O              # 160
KT = F // 128           # 72
S2 = 36                 # 6x6 conv2 positions per image
N2 = BL * S2
BCH = [(0, 14), (14, 14), (28, 14), (42, 14), (56, 8)]
NIT = 3


def _sub(ap, off, dims):
    """Arbitrary strided view (offset in elements, dims=[[step,count],..])."""
    return bass.AP(ap.tensor, ap.offset + off, [list(d) for d in dims])


def _pp(ap):
    """Partition pitch (elements per partition row) of an SBUF AP."""
    return ap.ap[0][0]


def build_nc(for_sim=False, reps=1):
    nc = bacc.Bacc("TRN2", target_bir_lowering=False, debug=False,
                   num_devices=1 if for_sim else NCORES)
    nc._for_sim = for_sim

    xin = nc.dram_tensor("xin", [BL * 784 + 8], BF16, kind="ExternalInput").ap()
    w1t = nc.dram_tensor("w1t", [81, 256], BF16, kind="ExternalInput").ap()
    b1 = nc.dram_tensor("b1", [128, 2], F32, kind="ExternalInput").ap()
    w2s = nc.dram_tensor("w2s", [162, 128, 256], BF16, kind="ExternalInput").ap()
    b2 = nc.dram_tensor("b2", [128, 2], F32, kind="ExternalInput").ap()
    wlb = nc.dram_tensor("wlb", [F, CO], BF16, kind="ExternalInput").ap()
    wtf = nc.dram_tensor("wtf", [CO, F], F32, kind="ExternalInput").ap()
    sel8 = nc.dram_tensor("sel8", [128, 8], F32, kind="ExternalInput").ap()
    sel2 = nc.dram_tensor("sel2", [32, 2], F32, kind="ExternalInput").ap()
    eyeb = nc.dram_tensor("eyeb", [64, 64], BF16, kind="ExternalInput").ap()
    eyef = nc.dram_tensor("eyef", [16, 16], F32, kind="ExternalInput").ap()
    out = nc.dram_tensor("out", [BL, CO], F32, kind="ExternalOutput").ap()

    selr = nc.dram_tensor("selr", [8, 128, 128], BF16, kind="ExternalInput").ap()
    cc_in = nc.dram_tensor("cc_in", [C, R], F32)
    cc_out = nc.dram_tensor("cc_out", [C, R], F32,
                            addr_space="Local" if for_sim else "Shared")
    vd = nc.dram_tensor("vd", [2, 128, N2], F32)       # conv2 out bounce

    with tile.TileContext(nc, num_cores=NCORES) as tc:
        for _rep in range(reps):
            _body(tc, nc, xin, w1t, b1, w2s, b2, wlb, wtf, sel8, sel2,
                  eyeb, eyef, selr, out, cc_in, cc_out, vd)
    nc.compile()
    return nc


def _body(tc, nc, xin, w1t, b1, w2s, b2, wlb, wtf, sel8, sel2, eyeb, eyef,
          selr, out, cc_in, cc_out, vd):
    with tc.tile_pool(name="const", bufs=1) as pc, \
         tc.tile_pool(name="upers", bufs=1) as pU:

        w1t_sb = pc.tile([81, 256], BF16, tag="w1t")
        nc.sync.dma_start(w1t_sb[:], w1t)
        b1_sb = pc.tile([128, 2], F32, tag="b1")
        nc.sync.dma_start(b1_sb[:], b1)
        b2_sb = pc.tile([128, 2], F32, tag="b2")
        nc.sync.dma_start(b2_sb[:], b2)
        sel8_sb = pc.tile([128, 8], F32, tag="sel8")
        nc.sync.dma_start(sel8_sb[:], sel8)
        sel2_sb = pc.tile([32, 2], F32, tag="sel2")
        nc.sync.dma_start(sel2_sb[:], sel2)
        eyeb_sb = pc.tile([64, 64], BF16, tag="eyeb")
        nc.sync.dma_start(eyeb_sb[:], eyeb)
        eyef_sb = pc.tile([16, 16], F32, tag="eyef")
        nc.sync.dma_start(eyef_sb[:], eyef)
        selr_sb = pc.tile([128, 8 * 128], BF16, tag="selr")
        nc.sync.dma_start(
            _sub(selr_sb[:], 0, [[_pp(selr_sb[:]), 128], [128, 8], [1, 128]]),
            _sub(selr, 0, [[128, 128], [128 * 128, 8], [1, 128]]))

        u2Tb = pU.tile([BL, F], BF16, tag="u2Tb")       # squashed u, b-major
        u2R = pU.tile([128, KT * BL], BF16, tag="u2R")  # squashed u, f-major

        # ============ Phase A: conv1 + conv2 + capsule formation ===========
        with tc.tile_pool(name="uT", bufs=1) as pT:
            u2T = pT.tile([BL, F], F32, tag="u2T")      # raw capsules, b-major

            with tc.tile_pool(name="pA", bufs=1) as pA, \
                 tc.tile_pool(name="pH", bufs=1) as pH, \
                 tc.tile_pool(name="pW2", bufs=8) as pW2, \
                 tc.tile_pool(name="pV", bufs=1) as pV, \
                 tc.tile_pool(name="ps1", bufs=2, space="PSUM") as ps1, \
                 tc.tile_pool(name="ps2", bufs=1, space="PSUM") as ps2:

                h1 = [pH.tile([128, BL * JC], BF16, tag=f"h1_{kc}",
                              name=f"h1_{kc}") for kc in range(2)]

                flip = 0
                for sb in range(NSB):
                    A = pA.tile([81, SB * J], BF16, tag="A")
                    pa = _pp(A[:])
                    for kh in range(9):
                        src = _sub(xin, sb * SB * 784 + 28 * kh,
                                   [[1, 9], [784, SB], [1, J]])
                        dst = _sub(A[:], 9 * kh * pa,
                                   [[pa, 9], [J, SB], [1, J]])
                        nc.sync.dma_start(dst, src)

                    for mc in range(2):
                        lhsT = w1t_sb[:, mc * 128:(mc + 1) * 128]
                        for bi in range(SB):
                            for hf in range(2):
                                ps = ps1.tile([128, 280], F32, tag="c1ps")
                                rhs = A[:, bi * J + hf * 280: bi * J + hf * 280 + 280]
                                nc.tensor.matmul(ps[:], lhsT, rhs,
                                                 start=True, stop=True)
                                doff = (sb * SB + bi) * JC + hf * 200
                                dstc = _sub(h1[mc][:], doff,
                                            [[_pp(h1[mc][:]), 128], [20, 10], [1, 20]])
                                srcc = _sub(ps[:], 0,
                                            [[_pp(ps[:]), 128], [28, 10], [1, 20]])
                                bb = b1_sb[:, mc:mc + 1]
                                if flip % 2 == 0:
                                    nc.vector.tensor_scalar(dstc, srcc, bb, 0.0,
                                                            op0=ADD, op1=MAX)
                                else:
                                    nc.scalar.activation(dstc, srcc, ACT.Relu,
                                                         bias=bb)
                                flip += 1

                # conv2
                for mc in range(2):
                    pss = [ps2.tile([128, nb * S2], F32, tag=f"c2ps{i}",
                                    name=f"c2ps{i}_{mc}")
                           for i, (_, nb) in enumerate(BCH)]
                    for kc in range(2):
                        for khw in range(81):
                            kh2, kw2 = khw // 9, khw % 9
                            wch = pW2.tile([128, 256], BF16, tag="wch")
                            nc.sync.dma_start(wch[:], w2s[khw * 2 + kc])
                            lhsT = wch[:, mc * 128:(mc + 1) * 128]
                            for ic, (b0, nb) in enumerate(BCH):
                                rhs = _sub(h1[kc][:], b0 * JC + 20 * kh2 + kw2,
                                           [[_pp(h1[kc][:]), 128],
                                            [JC, nb], [40, 6], [2, 6]])
                                nc.tensor.matmul(
                                    pss[ic][:], lhsT, rhs,
                                    start=(kc == 0 and khw == 0),
                                    stop=(kc == 1 and khw == 80))
                    v = pV.tile([128, N2], F32, tag="v")
                    for ic, (b0, nb) in enumerate(BCH):
                        nc.vector.tensor_scalar(v[:, b0 * S2:(b0 + nb) * S2],
                                                pss[ic][:], b2_sb[:, mc:mc + 1],
                                                None, op0=ADD)
                    # bounce via DRAM: SBUF-side DMA APs need the partition
                    # dim outermost, so the (co,b)->(b,co) transpose is done
                    # on the DRAM side
                    nc.sync.dma_start(vd.ap()[mc], v[:])
                    usrc = _sub(vd.ap(), mc * 128 * N2,
                                [[S2, BL], [N2, 128], [1, S2]])
                    udst = _sub(u2T[:], mc * 128 * S2,
                                [[_pp(u2T[:]), BL], [S2, 128], [1, S2]])
                    nc.sync.dma_start(udst, usrc)

            # ============ squash u (capsule groups of 8) ===================
            with tc.tile_pool(name="squ", bufs=1) as pq:
                sqr = pq.tile([BL, F], F32, tag="sqr")
                nc.vector.tensor_mul(sqr[:], u2T[:], u2T[:])
                sq = pq.tile([BL, R], F32, tag="sq")
                nc.vector.tensor_reduce(sq[:],
                                        sqr[:].rearrange("p (r i) -> p r i", i=I),
                                        axis=AXX, op=ADD)
                srt = pq.tile([BL, R], F32, tag="srt")
                nc.scalar.sqrt(srt[:], sq[:])
                d1 = pq.tile([BL, R], F32, tag="d1")
                nc.vector.tensor_scalar(d1[:], sq[:], 1.0, None, op0=ADD)
                d2 = pq.tile([BL, R], F32, tag="d2")
                nc.vector.tensor_mul(d2[:], d1[:], srt[:])
                rc = pq.tile([BL, R], F32, tag="rc")
                nc.vector.reciprocal(rc[:], d2[:])
                g = pq.tile([BL, R], F32, tag="g")
                nc.vector.tensor_mul(g[:], sq[:], rc[:])
                # u2Tb = u2T * g, one strided pass per capsule element
                ppu = _pp(u2T[:])
                ppb = _pp(u2Tb[:])
                for i in range(I):
                    nc.vector.tensor_tensor(
                        _sub(u2Tb[:], i, [[ppb, BL], [I, R]]),
                        _sub(u2T[:], i, [[ppu, BL], [I, R]]),
                        g[:], op=MUL)

        # ============ u2R = transpose(u2Tb) ================================
        with tc.tile_pool(name="ptr", bufs=2, space="PSUM") as ptr:
            for t in range(KT):
                pst = ptr.tile([128, BL], BF16, tag="tr")
                nc.tensor.transpose(pst[:], u2Tb[:, t * 128:(t + 1) * 128],
                                    eyeb_sb[:])
                nc.vector.tensor_copy(u2R[:, t * BL:(t + 1) * BL], pst[:])

        # ============ routing ==============================================
        with tc.tile_pool(name="pB", bufs=1) as pB, \
             tc.tile_pool(name="pBs", bufs=2) as pBs, \
             tc.tile_pool(name="psq2", bufs=1) as pq, \
             tc.tile_pool(name="psB", bufs=2, space="PSUM") as psB, \
             tc.tile_pool(name="psS", bufs=1, space="PSUM") as psS:

            wsb = pB.tile([128, KT * CO], BF16, tag="wsb")
            wsrc = _sub(wlb, 0, [[CO, 128], [128 * CO, KT], [1, CO]])
            wdst = _sub(wsb[:], 0, [[_pp(wsb[:]), 128], [CO, KT], [1, CO]])
            nc.sync.dma_start(wdst, wsrc)
            wt0 = pB.tile([128, F], F32, tag="wt0")
            nc.sync.dma_start(wt0[:], wtf[0:128])
            wt1 = pB.tile([32, F], F32, tag="wt1")
            nc.sync.dma_start(wt1[:], wtf[128:160])
            wp = pB.tile([128, KT * CO], BF16, tag="wp")
            cE = pB.tile([128, KT * C], BF16, tag="cE")
            cTr = pB.tile([128, 9 * C], BF16, tag="cTr")
            mAll = pB.tile([8, R], F32, tag="mAll")
            mAll2 = pB.tile([2, R], F32, tag="mAll2")
            bijA = pB.tile([C, R], F32, tag="bijA")
            bijB = pB.tile([C, R], F32, tag="bijB")
            csm = pB.tile([C, R], F32, tag="csm")
            v2T = pB.tile([BL, CO], F32, tag="v2T")
            v2Tb = pB.tile([BL, CO], BF16, tag="v2Tb")
            msum = pB.tile([C, R], F32, tag="msum")

            lam = 1.0 / R
            for it in range(NIT):
                if it > 0:
                    # cTr[r%128, q*10+c] = csm[c, r]  (PE transpose, 9 blocks)
                    for q in range(9):
                        pst = psB.tile([128, C], F32, tag="ctr", name="ctr", bufs=1)
                        nc.tensor.transpose(pst[:],
                                            csm[:, q * 128:(q + 1) * 128],
                                            eyef_sb[0:C, 0:C])
                        nc.vector.tensor_copy(cTr[:, q * C:(q + 1) * C], pst[:])
                    # cE[8r''+i, (8t2+t1)*10+c] = cTr[16*t1+r'', t2*10+c]
                    # via selector matmuls: SEL_t1[k, p] = (k == 16*t1 + p//8)
                    for t1 in range(8):
                        pse = psB.tile([128, 9 * C], F32, tag="cexp", name="cexp", bufs=1)
                        nc.tensor.matmul(pse[:],
                                         selr_sb[:, t1 * 128:(t1 + 1) * 128],
                                         cTr[:], start=True, stop=True)
                        nc.vector.tensor_copy(
                            _sub(cE[:], t1 * C,
                                 [[_pp(cE[:]), 128], [8 * C, 9], [1, C]]),
                            pse[:])
                    # wp = wsb * cE, one strided pass per o
                    ppw = _pp(wp[:])
                    pps = _pp(wsb[:])
                    for o in range(O):
                        nc.vector.tensor_tensor(
                            _sub(wp[:], o, [[ppw, 128], [CO, KT], [O, C]]),
                            _sub(wsb[:], o, [[pps, 128], [CO, KT], [O, C]]),
                            cE[:].rearrange("p (t c) -> p t c", c=C), op=MUL)

                # s_j^T [b, co] over 72 accumulating K-tiles
                wcur = wsb if it == 0 else wp
                ssum = psS.tile([BL, CO], F32, tag="ssum")
                for t in range(KT):
                    nc.tensor.matmul(ssum[:], u2R[:, t * BL:(t + 1) * BL],
                                     wcur[:, t * CO:(t + 1) * CO],
                                     start=(t == 0), stop=(t == KT - 1))

                # v2 = squash(s) over o-groups of 16 (iter0 folds the 1/R scale)
                ssb = pq.tile([BL, CO], F32, tag="ssb")
                nc.vector.tensor_copy(ssb[:], ssum[:])
                svr = pq.tile([BL, CO], F32, tag="svr")
                nc.vector.tensor_mul(svr[:], ssb[:], ssb[:])
                sqv = pq.tile([BL, C], F32, tag="sqv")
                nc.vector.tensor_reduce(sqv[:],
                                        svr[:].rearrange("p (c o) -> p c o", o=O),
                                        axis=AXX, op=ADD)
                if it == 0:
                    nc.vector.tensor_scalar(sqv[:], sqv[:], lam * lam, None, op0=MUL)
                srtv = pq.tile([BL, C], F32, tag="srtv")
                nc.scalar.sqrt(srtv[:], sqv[:])
                dv1 = pq.tile([BL, C], F32, tag="dv1")
                nc.vector.tensor_scalar(dv1[:], sqv[:], 1.0, None, op0=ADD)
                dv2 = pq.tile([BL, C], F32, tag="dv2")
                nc.vector.tensor_mul(dv2[:], dv1[:], srtv[:])
                rcv = pq.tile([BL, C], F32, tag="rcv")
                nc.vector.reciprocal(rcv[:], dv2[:])
                gv = pq.tile([BL, C], F32, tag="gv")
                nc.vector.tensor_mul(gv[:], sqv[:], rcv[:])
                if it == 0:
                    nc.vector.tensor_scalar(gv[:], gv[:], lam, None, op0=MUL)
                ppv = _pp(v2T[:])
                pps2 = _pp(ssb[:])
                for o in range(O):
                    nc.vector.tensor_tensor(
                        _sub(v2T[:], o, [[ppv, BL], [O, C]]),
                        _sub(ssb[:], o, [[pps2, BL], [O, C]]),
                        gv[:], op=MUL)

                if it == NIT - 1:
                    nc.sync.dma_start(out, v2T[:])
                    break

                nc.vector.tensor_copy(v2Tb[:], v2T[:])
                # m[c, r] = sum_{o,i} Wt[(c,o),(r,i)] * (v2^T @ u2)[(c,o),(r,i)]
                for mc2 in range(2):
                    npart = 128 if mc2 == 0 else 32
                    ncls = 8 if mc2 == 0 else 2
                    lhs = v2Tb[:, mc2 * 128: mc2 * 128 + npart]
                    selt = (sel8_sb if mc2 == 0 else sel2_sb)[0:npart, 0:ncls]
                    wtt = wt0 if mc2 == 0 else wt1
                    for nch in range(18):
                        f0 = nch * 512
                        tps = psB.tile([128, 512], F32, tag="tprime")
                        nc.tensor.matmul(tps[0:npart, :], lhs,
                                         u2Tb[:, f0:f0 + 512],
                                         start=True, stop=True)
                        pm = pBs.tile([128, 512], F32, tag="pm")
                        nc.vector.tensor_tensor(pm[0:npart, :],
                                                wtt[0:npart, f0:f0 + 512],
                                                tps[0:npart, :], op=MUL)
                        pr = pBs.tile([128, 64], F32, tag="pr")
                        nc.vector.tensor_reduce(
                            pr[0:npart, :],
                            pm[0:npart, :].rearrange("p (r i) -> p r i", i=I),
                            axis=AXX, op=ADD)
                        mo = psB.tile([16, 64], F32, tag="mo", bufs=2)
                        nc.tensor.matmul(mo[0:ncls, :], selt, pr[0:npart, :],
                                         start=True, stop=True)
                        mtgt = mAll if mc2 == 0 else mAll2
                        nc.vector.tensor_copy(
                            mtgt[0:ncls, f0 // I: f0 // I + 64],
                            mo[0:ncls, :])

                nc.sync.dma_start(cc_in.ap()[0:8], mAll[:])
                nc.sync.dma_start(cc_in.ap()[8:10], mAll2[:])
                if getattr(nc, "_for_sim", False):
                    nc.sync.dma_start(cc_out.ap(), cc_in.ap())
                else:
                    nc.gpsimd.collective_compute(
                        "AllReduce", ADD,
                        replica_groups=[list(range(NCORES))],
                        ins=[cc_in.ap()], outs=[cc_out.ap()])
                nc.sync.dma_start(msum[:], cc_out.ap())
                bij = bijA if it == 0 else bijB
                if it == 0:
                    nc.vector.tensor_scalar(bij[:], msum[:], 1.0 / B, None, op0=MUL)
                else:
                    nc.vector.tensor_scalar(bij[:], msum[:], 1.0 / B, None, op0=MUL)
                    nc.vector.tensor_add(bij[:], bij[:], bijA[:])
                # softmax over routes (free dim)
                rmax = pq.tile([C, 1], F32, tag="rmax")
                nc.vector.tensor_reduce(rmax[:], bij[:], axis=AXX, op=MAX)
                nrm = pq.tile([C, 1], F32, tag="nrm")
                nc.vector.tensor_scalar(nrm[:], rmax[:], -1.0, None, op0=MUL)
                nc.scalar.activation(csm[:], bij[:], ACT.Exp, bias=nrm[:])
                rsm = pq.tile([C, 1], F32, tag="rsm")
                nc.vector.tensor_reduce(rsm[:], csm[:], axis=AXX, op=ADD)
                rrc = pq.tile([C, 1], F32, tag="rrc")
                nc.vector.reciprocal(rrc[:], rsm[:])
                nc.vector.tensor_scalar(csm[:], csm[:], rrc[:], None, op0=MUL)


# ------------------------- host side ---------------------------------------
_CACHE = {}


def kernel(x, conv1_w, conv1_b, conv2_w, conv2_b, W):
    if "nc" not in _CACHE:
        _CACHE["nc"] = build_nc()
    nc = _CACHE["nc"]

    bf = ml_dtypes.bfloat16
    xf = np.ascontiguousarray(np.asarray(x, np.float32).reshape(B, 784))
    w1 = np.ascontiguousarray(
        np.asarray(conv1_w, np.float32).reshape(256, 81).T).astype(bf)
    b1v = np.asarray(conv1_b, np.float32).reshape(2, 128).T.copy()
    w2 = np.asarray(conv2_w, np.float32).reshape(256, 256, 81)
    w2 = np.ascontiguousarray(w2.transpose(2, 1, 0)).reshape(162, 128, 256).astype(bf)
    b2v = np.asarray(conv2_b, np.float32).reshape(2, 128).T.copy()
    Wf = np.asarray(W, np.float32)
    wl = np.ascontiguousarray(Wf.transpose(0, 3, 1, 2)).reshape(F, CO).astype(bf)
    wt = np.ascontiguousarray(Wf.transpose(1, 2, 0, 3)).reshape(CO, F).astype(np.float32)
    s8 = np.zeros((128, 8), np.float32)
    s8[np.arange(128), np.arange(128) // 16] = 1.0
    s2m = np.zeros((32, 2), np.float32)
    s2m[np.arange(32), np.arange(32) // 16] = 1.0
    srn = np.zeros((8, 128, 128), np.float32)
    for t1 in range(8):
        srn[t1, 16 * t1 + np.arange(128) // 8, np.arange(128)] = 1.0

    shared = {
        "w1t": w1, "b1": b1v, "w2s": w2, "b2": b2v, "wlb": wl, "wtf": wt,
        "sel8": s8, "sel2": s2m, "selr": srn.astype(bf),
        "eyeb": np.eye(64).astype(bf), "eyef": np.eye(16, dtype=np.float32),
    }
    in_maps = []
    for c in range(NCORES):
        xs = np.zeros(BL * 784 + 8, bf)
        xs[:BL * 784] = xf[c * BL:(c + 1) * BL].reshape(-1).astype(bf)
        in_maps.append({"xin": xs, **shared})
    res = run_bass_kernel_spmd(nc, in_maps, list(range(NCORES)), trace=False)
    outs = [res.results[c]["out"] for c in range(NCORES)]
    return np.concatenate(outs, axis=0).reshape(B, C, O).astype(np.float32)



# revision 9
# speedup vs baseline: 32.3238x; 32.3238x over previous
"""CapsuleNet forward kernel for 8 Trainium2 NeuronCores (v2).

Data-parallel over batch (64 images / core); the routing b_ij batch-mean
uses an AllReduce per iteration.  u_hat is never materialized: s_j and the
agreement mean are computed directly against W from the 9216-dim flattened
capsule vector u.

v2 layout notes (vs v1):
  h1 stored b-innermost  h1[ci, row*1280 + col*64 + b]  so conv2's moving
  operand streams 64-element contiguous runs (full PE rate; the v1
  [40,6],[2,6] pattern ran at 0.60 ns/row vs 0.42 ideal).
  conv2 psum is 6 y-banks of [co, x*64+b]; its free dim is s*64+b, so the
  co-major -> b-major conversion is 18 PE transposes per mc chunk plus
  strided psum->sbuf copies (no DRAM bounce).
  squash + u2R transposes run per mc-half so half 0 overlaps conv2 mc1.
  routing works in bf16 with (o,c)-ordered W so the c_ij scaling is one
  broadcast-stride tensor_tensor, and W (.) (v^T u) uses 2x-mode DVE ops.
"""

import numpy as np
import ml_dtypes

import concourse.bacc as bacc
import concourse.bass as bass
import concourse.mybir as mybir
import concourse.tile as tile
from concourse.bass_utils import run_bass_kernel_spmd

F32 = mybir.dt.float32
BF16 = mybir.dt.bfloat16
MUL = mybir.AluOpType.mult
ADD = mybir.AluOpType.add
MAX = mybir.AluOpType.max
AXX = mybir.AxisListType.X
ACT = mybir.ActivationFunctionType

NCORES = 8
B = 512
BL = B // NCORES        # 64 images per core
SB = 16                 # conv1 im2col sub-batch
NSB = BL // SB
J = 560                 # 20 rows x 28 cols (8 garbage cols/row)
R, C, O, I = 1152, 10, 16, 8
F = R * I               # 9216
CO = C * O              # 160
KT = F // 128           # 72
S2 = 36                 # 6x6 conv2 positions per image
N2 = BL * S2            # 2304
HB = 1280               # h1 row stride (20 cols * 64 b)
NIT = 3
FH = F // 2             # 4608 per half
RH = R // 2             # 576


def _sub(ap, off, dims):
    """Arbitrary strided view (offset in elements, dims=[[step,count],..])."""
    return bass.AP(ap.tensor, ap.offset + off, [list(d) for d in dims])


def _pp(ap):
    """Partition pitch (elements per partition row) of an SBUF AP."""
    return ap.ap[0][0]


def build_nc(for_sim=False, reps=1):
    nc = bacc.Bacc("TRN2", target_bir_lowering=False, debug=False,
                   num_devices=1 if for_sim else NCORES)
    nc._for_sim = for_sim

    xin = nc.dram_tensor("xin", [BL * 784 + 8], BF16, kind="ExternalInput").ap()
    w1t = nc.dram_tensor("w1t", [81, 256], BF16, kind="ExternalInput").ap()
    b1 = nc.dram_tensor("b1", [128, 2], F32, kind="ExternalInput").ap()
    w2s = nc.dram_tensor("w2s", [162, 128, 256], BF16, kind="ExternalInput").ap()
    b2 = nc.dram_tensor("b2", [128, 2], F32, kind="ExternalInput").ap()
    wsb_d = nc.dram_tensor("wsb_d", [F, CO], BF16, kind="ExternalInput").ap()
    wtf_d = nc.dram_tensor("wtf_d", [CO, F], BF16, kind="ExternalInput").ap()
    selt_d = nc.dram_tensor("selt_d", [CO, C], BF16, kind="ExternalInput").ap()
    eye_d = nc.dram_tensor("eye_d", [128, 128], BF16, kind="ExternalInput").ap()
    selr = nc.dram_tensor("selr", [8, 128, 128], BF16, kind="ExternalInput").ap()
    out = nc.dram_tensor("out", [BL, CO], F32, kind="ExternalOutput").ap()

    cc_in = nc.dram_tensor("cc_in", [C, R], F32)
    cc_out = nc.dram_tensor("cc_out", [C, R], F32,
                            addr_space="Local" if for_sim else "Shared")

    with tile.TileContext(nc, num_cores=NCORES) as tc:
        for _rep in range(reps):
            _body(tc, nc, xin, w1t, b1, w2s, b2, wsb_d, wtf_d, selt_d,
                  eye_d, selr, out, cc_in, cc_out)
    nc.compile()
    return nc


def _body(tc, nc, xin, w1t, b1, w2s, b2, wsb_d, wtf_d, selt_d, eye_d,
          selr, out, cc_in, cc_out):
    with tc.tile_pool(name="const", bufs=1) as pc, \
         tc.tile_pool(name="upers", bufs=1) as pU:

        w1t_sb = pc.tile([81, 256], BF16, tag="w1t")
        nc.sync.dma_start(w1t_sb[:], w1t)
        b1_sb = pc.tile([128, 2], F32, tag="b1")
        nc.sync.dma_start(b1_sb[:], b1)
        b2_sb = pc.tile([128, 2], F32, tag="b2")
        nc.sync.dma_start(b2_sb[:], b2)
        eye_sb = pc.tile([128, 128], BF16, tag="eye")
        nc.sync.dma_start(eye_sb[:], eye_d)
        selt_sb = pc.tile([128, C], BF16, tag="selt")
        nc.sync.dma_start(selt_sb[:], selt_d[0:128])
        selt2_sb = pc.tile([32, C], BF16, tag="selt2")
        nc.sync.dma_start(selt2_sb[:], selt_d[128:160])
        selr_sb = pc.tile([128, 8 * 128], BF16, tag="selr")
        nc.sync.dma_start(
            _sub(selr_sb[:], 0, [[_pp(selr_sb[:]), 128], [128, 8], [1, 128]]),
            _sub(selr, 0, [[128, 128], [128 * 128, 8], [1, 128]]))

        u2Tb = pU.tile([BL, F], BF16, tag="u2Tb")       # squashed u, b-major
        u2R = pU.tile([128, KT * BL], BF16, tag="u2R")  # squashed u, f-major

        # ============ Phase A: conv1 + conv2 + capsule formation ===========
        with tc.tile_pool(name="pT", bufs=1) as pT, \
             tc.tile_pool(name="pH", bufs=1) as pH, \
             tc.tile_pool(name="pW2", bufs=8) as pW2, \
             tc.tile_pool(name="pV", bufs=1) as pV, \
             tc.tile_pool(name="ps2", bufs=1, space="PSUM") as ps2:

            u2T = pT.tile([BL, F], BF16, tag="u2T")     # raw capsules, b-major
            h1 = [pH.tile([128, 20 * HB], BF16, tag=f"h1_{kc}",
                          name=f"h1_{kc}") for kc in range(2)]

            # ---- conv1: [64,1,28,28] -> h1[ci, row*1280 + col*64 + b] ----
            with tc.tile_pool(name="pA", bufs=2) as pA, \
                 tc.tile_pool(name="ps1", bufs=2, space="PSUM") as ps1:
                flip = 0
                for sb in range(NSB):
                    A = pA.tile([81, SB * J], BF16, tag="A")
                    pa = _pp(A[:])
                    for kh in range(9):
                        src = _sub(xin, sb * SB * 784 + 28 * kh,
                                   [[1, 9], [784, SB], [1, J]])
                        dst = _sub(A[:], 9 * kh * pa,
                                   [[pa, 9], [J, SB], [1, J]])
                        nc.sync.dma_start(dst, src)

                    for mc in range(2):
                        lhsT = w1t_sb[:, mc * 128:(mc + 1) * 128]
                        for bi in range(SB):
                            for hf in range(2):
                                ps = ps1.tile([128, 280], F32, tag="c1ps")
                                rhs = A[:, bi * J + hf * 280: bi * J + hf * 280 + 280]
                                nc.tensor.matmul(ps[:], lhsT, rhs,
                                                 start=True, stop=True)
                                b = sb * SB + bi
                                dstc = _sub(h1[mc][:], hf * 10 * HB + b,
                                            [[_pp(h1[mc][:]), 128],
                                             [HB, 10], [64, 20]])
                                srcc = _sub(ps[:], 0,
                                            [[_pp(ps[:]), 128], [28, 10], [1, 20]])
                                bb = b1_sb[:, mc:mc + 1]
                                if flip % 2 == 0:
                                    nc.vector.tensor_scalar(dstc, srcc, bb, 0.0,
                                                            op0=ADD, op1=MAX)
                                else:
                                    nc.scalar.activation(dstc, srcc, ACT.Relu,
                                                         bias=bb)
                                flip += 1

            # ---- conv2 + capsule transposes + squash, per mc half ----
            with tc.tile_pool(name="paux", bufs=2, space="PSUM") as paux, \
                 tc.tile_pool(name="sq", bufs=1) as pq:

                sqr = pq.tile([BL, FH], BF16, tag="sqr")
                sq = pq.tile([BL, R], F32, tag="sq")
                srt = pq.tile([BL, R], F32, tag="srt")
                d1 = pq.tile([BL, R], F32, tag="d1")
                d2 = pq.tile([BL, R], F32, tag="d2")
                rc = pq.tile([BL, R], F32, tag="rc")
                g = pq.tile([BL, R], F32, tag="g")
                gb = pq.tile([BL, R], BF16, tag="gb")

                for mc in range(2):
                    pss = [ps2.tile([128, 6 * BL], F32, tag=f"c2ps{y}",
                                    name=f"c2ps{y}_{mc}")
                           for y in range(6)]
                    for kc in range(2):
                        for khw in range(81):
                            kh2, kw2 = khw // 9, khw % 9
                            wch = pW2.tile([128, 256], BF16, tag="wch")
                            nc.sync.dma_start(wch[:], w2s[khw * 2 + kc])
                            lhsT = wch[:, mc * 128:(mc + 1) * 128]
                            for y in range(6):
                                rhs = _sub(h1[kc][:],
                                           (2 * y + kh2) * HB + kw2 * 64,
                                           [[_pp(h1[kc][:]), 128],
                                            [128, 6], [1, 64]])
                                nc.tensor.matmul(
                                    pss[y][:], lhsT, rhs,
                                    start=(kc == 0 and khw == 0),
                                    stop=(kc == 1 and khw == 80))
                    # bias add on Act: v[co, s*64+b] bf16
                    v = pV.tile([128, N2], BF16, tag="v")
                    for y in range(6):
                        nc.vector.tensor_scalar(v[:, y * 6 * BL:(y + 1) * 6 * BL],
                                                pss[y][:], b2_sb[:, mc:mc + 1],
                                                None, op0=ADD)
                    # co-major -> b-major via PE transposes (no DRAM bounce):
                    # chunk c covers s-pair (2c, 2c+1); out partition
                    # j = (s%2)*64 + b, free = co.
                    ppu = _pp(u2T[:])
                    for ch in range(18):
                        tch = paux.tile([128, 128], BF16, tag="aux", name="tch")
                        nc.tensor.transpose(tch[:], v[:, ch * 128:(ch + 1) * 128],
                                            eye_sb[:])
                        for half in range(2):
                            s = 2 * ch + half
                            dst = _sub(u2T[:], mc * 128 * S2 + s,
                                       [[ppu, BL], [S2, 128]])
                            src = tch[half * 64:(half + 1) * 64, :]
                            if (ch + half) % 2 == 0:
                                nc.vector.tensor_copy(dst, src)
                            else:
                                nc.scalar.activation(dst, src, ACT.Copy)

                    # ---- squash of this half (f in [mc*FH, mc*FH+FH)) ----
                    f0, r0 = mc * FH, mc * RH
                    uh = u2T[:, f0:f0 + FH]
                    nc.vector.tensor_mul(sqr[:], uh, uh)
                    nc.vector.tensor_reduce(
                        sq[:, r0:r0 + RH],
                        sqr[:].rearrange("p (r i) -> p r i", i=I),
                        axis=AXX, op=ADD)
                    nc.scalar.sqrt(srt[:, r0:r0 + RH], sq[:, r0:r0 + RH])
                    nc.vector.tensor_scalar(d1[:, r0:r0 + RH],
                                            sq[:, r0:r0 + RH], 1.0, None, op0=ADD)
                    nc.vector.tensor_mul(d2[:, r0:r0 + RH],
                                         d1[:, r0:r0 + RH], srt[:, r0:r0 + RH])
                    nc.vector.reciprocal(rc[:, r0:r0 + RH], d2[:, r0:r0 + RH])
                    nc.vector.tensor_mul(g[:, r0:r0 + RH],
                                         sq[:, r0:r0 + RH], rc[:, r0:r0 + RH])
                    nc.vector.tensor_copy(gb[:, r0:r0 + RH], g[:, r0:r0 + RH])
                    ppb = _pp(u2Tb[:])
                    for i in range(I):
                        nc.vector.tensor_tensor(
                            _sub(u2Tb[:], f0 + i, [[ppb, BL], [I, RH]]),
                            _sub(u2T[:], f0 + i, [[ppu, BL], [I, RH]]),
                            gb[:, r0:r0 + RH], op=MUL)
                    # ---- u2R chunks of this half ----
                    for t in range(mc * (KT // 2), (mc + 1) * (KT // 2)):
                        pst = paux.tile([128, 128], BF16, tag="aux", name="pst")
                        nc.tensor.transpose(pst[:, 0:BL],
                                            u2Tb[:, t * 128:(t + 1) * 128],
                                            eye_sb[0:BL, 0:BL])
                        if t % 2 == 0:
                            nc.vector.tensor_copy(u2R[:, t * BL:(t + 1) * BL],
                                                  pst[:, 0:BL])
                        else:
                            nc.scalar.activation(u2R[:, t * BL:(t + 1) * BL],
                                                 pst[:, 0:BL], ACT.Copy)

        # ============ routing ==============================================
        with tc.tile_pool(name="pB", bufs=1) as pB, \
             tc.tile_pool(name="pBs", bufs=2) as pBs, \
             tc.tile_pool(name="psq2", bufs=1) as pq, \
             tc.tile_pool(name="psB", bufs=2, space="PSUM") as psB, \
             tc.tile_pool(name="psS", bufs=1, space="PSUM") as psS:

            # W in (t, o, c) order for ssum; (o,c)-major for the agreement
            wsb = pB.tile([128, KT * CO], BF16, tag="wsb")
            wsrc = _sub(wsb_d, 0, [[CO, 128], [128 * CO, KT], [1, CO]])
            wdst = _sub(wsb[:], 0, [[_pp(wsb[:]), 128], [CO, KT], [1, CO]])
            nc.sync.dma_start(wdst, wsrc)
            wt0 = pB.tile([128, F], BF16, tag="wt0")
            nc.sync.dma_start(wt0[:], wtf_d[0:128])
            wt1 = pB.tile([32, F], BF16, tag="wt1")
            nc.sync.dma_start(wt1[:], wtf_d[128:160])
            wp = pB.tile([128, KT * CO], BF16, tag="wp")
            cE = pB.tile([128, KT * C], BF16, tag="cE")
            cTr = pB.tile([128, 9 * C], BF16, tag="cTr")
            mAll = pB.tile([C, R], F32, tag="mAll")
            bijA = pB.tile([C, R], F32, tag="bijA")
            bijB = pB.tile([C, R], F32, tag="bijB")
            csm = pB.tile([C, R], BF16, tag="csm")
            v2T = pB.tile([BL, CO], F32, tag="v2T")
            v2Tb = pB.tile([BL, CO], BF16, tag="v2Tb")
            msum = pB.tile([C, R], F32, tag="msum")

            lam = 1.0 / R
            for it in range(NIT):
                if it > 0:
                    # cTr[r%128, q*10+c] = csm[c, r]  (PE transpose, 9 blocks)
                    for q in range(9):
                        pst = psB.tile([128, C], BF16, tag="ctr", name="ctr",
                                       bufs=1)
                        nc.tensor.transpose(pst[:],
                                            csm[:, q * 128:(q + 1) * 128],
                                            eye_sb[0:C, 0:C])
                        nc.vector.tensor_copy(cTr[:, q * C:(q + 1) * C], pst[:])
                    # cE[8r''+i, (8t2+t1)*10+c] = cTr[16*t1+r'', t2*10+c]
                    for t1 in range(8):
                        pse = psB.tile([128, 9 * C], F32, tag="cexp",
                                       name="cexp", bufs=1)
                        nc.tensor.matmul(pse[:],
                                         selr_sb[:, t1 * 128:(t1 + 1) * 128],
                                         cTr[:], start=True, stop=True)
                        nc.vector.tensor_copy(
                            _sub(cE[:], t1 * C,
                                 [[_pp(cE[:]), 128], [8 * C, 9], [1, C]]),
                            pse[:])
                    # wp[p,(t,o,c)] = wsb[p,(t,o,c)] * cE[p,(t,c)]  (bcast o)
                    ppw, pps, ppc = _pp(wp[:]), _pp(wsb[:]), _pp(cE[:])
                    for o in range(O):
                        nc.vector.tensor_tensor(
                            _sub(wp[:], o * C, [[ppw, 128], [CO, KT], [1, C]]),
                            _sub(wsb[:], o * C, [[pps, 128], [CO, KT], [1, C]]),
                            _sub(cE[:], 0, [[ppc, 128], [C, KT], [1, C]]),
                            op=MUL)

                # s_j^T [b, (o,c)] over 72 accumulating K-tiles
                wcur = wsb if it == 0 else wp
                ssum = psS.tile([BL, CO], F32, tag="ssum")
                for t in range(KT):
                    nc.tensor.matmul(ssum[:], u2R[:, t * BL:(t + 1) * BL],
                                     wcur[:, t * CO:(t + 1) * CO],
                                     start=(t == 0), stop=(t == KT - 1))

                # v2 = squash(s) over o-groups (iter0 folds the 1/R scale)
                ssb = pq.tile([BL, CO], F32, tag="ssb")
                nc.vector.tensor_copy(ssb[:], ssum[:])
                svr = pq.tile([BL, CO], F32, tag="svr")
                nc.vector.tensor_mul(svr[:], ssb[:], ssb[:])
                sqv = pq.tile([BL, C], F32, tag="sqv")
                nc.vector.tensor_reduce(
                    sqv[:],
                    _sub(svr[:], 0, [[_pp(svr[:]), BL], [1, C], [C, O]]),
                    axis=AXX, op=ADD)
                if it == 0:
                    nc.vector.tensor_scalar(sqv[:], sqv[:], lam * lam, None,
                                            op0=MUL)
                srtv = pq.tile([BL, C], F32, tag="srtv")
                nc.scalar.sqrt(srtv[:], sqv[:])
                dv1 = pq.tile([BL, C], F32, tag="dv1")
                nc.vector.tensor_scalar(dv1[:], sqv[:], 1.0, None, op0=ADD)
                dv2 = pq.tile([BL, C], F32, tag="dv2")
                nc.vector.tensor_mul(dv2[:], dv1[:], srtv[:])
                rcv = pq.tile([BL, C], F32, tag="rcv")
                nc.vector.reciprocal(rcv[:], dv2[:])
                gv = pq.tile([BL, C], F32, tag="gv")
                nc.vector.tensor_mul(gv[:], sqv[:], rcv[:])
                if it == 0:
                    nc.vector.tensor_scalar(gv[:], gv[:], lam, None, op0=MUL)

                if it == NIT - 1:
                    # v2T in (c,o) order for the final output
                    nc.vector.tensor_tensor(
                        _sub(v2T[:], 0, [[_pp(v2T[:]), BL], [1, O], [O, C]]),
                        _sub(ssb[:], 0, [[_pp(ssb[:]), BL], [C, O], [1, C]]),
                        _sub(gv[:], 0, [[_pp(gv[:]), BL], [0, O], [1, C]]),
                        op=MUL)
                    nc.sync.dma_start(out, v2T[:])
                    break

                # v2T in (o,c) order; gv broadcast over o
                nc.vector.tensor_tensor(
                    _sub(v2T[:], 0, [[_pp(v2T[:]), BL], [C, O], [1, C]]),
                    _sub(ssb[:], 0, [[_pp(ssb[:]), BL], [C, O], [1, C]]),
                    _sub(gv[:], 0, [[_pp(gv[:]), BL], [0, O], [1, C]]),
                    op=MUL)
                nc.vector.tensor_copy(v2Tb[:], v2T[:])

                # m[c, r] = sum_{o,i} Wt[(o,c),(r,i)] * (v2^T @ u2)[(o,c),(r,i)]
                for nch in range(18):
                    f0 = nch * 512
                    tps = psB.tile([128, 512], F32, tag="tprime")
                    nc.tensor.matmul(tps[:], v2Tb[:, 0:128],
                                     u2Tb[:, f0:f0 + 512],
                                     start=True, stop=True)
                    tps2 = psB.tile([32, 512], F32, tag="tprime2", bufs=2)
                    nc.tensor.matmul(tps2[:], v2Tb[:, 128:160],
                                     u2Tb[:, f0:f0 + 512],
                                     start=True, stop=True)
                    tpb = pBs.tile([128, 512], BF16, tag="tpb")
                    nc.scalar.activation(tpb[:], tps[:], ACT.Copy)
                    tpb2 = pBs.tile([32, 512], BF16, tag="tpb2")
                    nc.scalar.activation(tpb2[:], tps2[:], ACT.Copy)
                    pm = pBs.tile([128, 512], BF16, tag="pm")
                    nc.vector.tensor_tensor(pm[:], wt0[:, f0:f0 + 512],
                                            tpb[:], op=MUL)
                    pm2 = pBs.tile([32, 512], BF16, tag="pm2")
                    nc.vector.tensor_tensor(pm2[:], wt1[:, f0:f0 + 512],
                                            tpb2[:], op=MUL)
                    pr = pBs.tile([128, 64], BF16, tag="pr")
                    pr2 = pBs.tile([32, 64], BF16, tag="pr2")
                    with nc.allow_low_precision(reason="agreement partial sums"):
                        nc.vector.tensor_reduce(
                            pr[:], pm[:].rearrange("p (r i) -> p r i", i=I),
                            axis=AXX, op=ADD)
                        nc.vector.tensor_reduce(
                            pr2[:], pm2[:].rearrange("p (r i) -> p r i", i=I),
                            axis=AXX, op=ADD)
                    mo = psB.tile([16, 64], F32, tag="mo", bufs=1)
                    nc.tensor.matmul(mo[0:C, :], selt_sb[:], pr[:],
                                     start=True, stop=False)
                    nc.tensor.matmul(mo[0:C, :], selt2_sb[:], pr2[:],
                                     start=False, stop=True)
                    nc.vector.tensor_copy(
                        mAll[:, f0 // I: f0 // I + 64], mo[0:C, :])

                nc.sync.dma_start(cc_in.ap(), mAll[:])
                if getattr(nc, "_for_sim", False):
                    nc.sync.dma_start(cc_out.ap(), cc_in.ap())
                else:
                    nc.gpsimd.collective_compute(
                        "AllReduce", ADD,
                        replica_groups=[list(range(NCORES))],
                        ins=[cc_in.ap()], outs=[cc_out.ap()])
                nc.sync.dma_start(msum[:], cc_out.ap())
                bij = bijA if it == 0 else bijB
                nc.vector.tensor_scalar(bij[:], msum[:], 1.0 / B, None, op0=MUL)
                if it > 0:
                    nc.vector.tensor_add(bij[:], bij[:], bijA[:])
                # softmax over routes (free dim), output bf16
                rmax = pq.tile([C, 1], F32, tag="rmax")
                nc.vector.tensor_reduce(rmax[:], bij[:], axis=AXX, op=MAX)
                nrm = pq.tile([C, 1], F32, tag="nrm")
                nc.vector.tensor_scalar(nrm[:], rmax[:], -1.0, None, op0=MUL)
                nc.scalar.activation(csm[:], bij[:], ACT.Exp, bias=nrm[:])
                rsm = pq.tile([C, 1], F32, tag="rsm")
                nc.vector.tensor_reduce(rsm[:], csm[:], axis=AXX, op=ADD)
                rrc = pq.tile([C, 1], F32, tag="rrc")
                nc.vector.reciprocal(rrc[:], rsm[:])
                nc.vector.tensor_scalar(csm[:], csm[:], rrc[:], None, op0=MUL)


# ------------------------- host side ---------------------------------------
_CACHE = {}


def pack_shared(conv1_w, conv1_b, conv2_w, conv2_b, W):
    bf = ml_dtypes.bfloat16
    w1 = np.ascontiguousarray(
        np.asarray(conv1_w, np.float32).reshape(256, 81).T).astype(bf)
    b1v = np.asarray(conv1_b, np.float32).reshape(2, 128).T.copy()
    w2 = np.asarray(conv2_w, np.float32).reshape(256, 256, 81)
    w2 = np.ascontiguousarray(w2.transpose(2, 1, 0)).reshape(162, 128, 256).astype(bf)
    b2v = np.asarray(conv2_b, np.float32).reshape(2, 128).T.copy()
    Wf = np.asarray(W, np.float32)                      # [R, C, O, I]
    wsb = np.ascontiguousarray(
        Wf.transpose(0, 3, 2, 1)).reshape(F, CO).astype(bf)      # [f,(o,c)]
    wtf = np.ascontiguousarray(
        Wf.transpose(2, 1, 0, 3)).reshape(CO, F).astype(bf)      # [(o,c),f]
    selt = np.zeros((CO, C), np.float32)
    selt[np.arange(CO), np.arange(CO) % C] = 1.0
    srn = np.zeros((8, 128, 128), np.float32)
    for t1 in range(8):
        srn[t1, 16 * t1 + np.arange(128) // 8, np.arange(128)] = 1.0
    return {
        "w1t": w1, "b1": b1v, "w2s": w2, "b2": b2v,
        "wsb_d": wsb, "wtf_d": wtf, "selt_d": selt.astype(bf),
        "selr": srn.astype(bf), "eye_d": np.eye(128).astype(bf),
    }


def make_in_maps(x, shared):
    bf = ml_dtypes.bfloat16
    xf = np.ascontiguousarray(np.asarray(x, np.float32).reshape(B, 784))
    in_maps = []
    for c in range(NCORES):
        xs = np.zeros(BL * 784 + 8, bf)
        xs[:BL * 784] = xf[c * BL:(c + 1) * BL].reshape(-1).astype(bf)
        in_maps.append({"xin": xs, **shared})
    return in_maps


def kernel(x, conv1_w, conv1_b, conv2_w, conv2_b, W):
    if "nc" not in _CACHE:
        _CACHE["nc"] = build_nc()
    nc = _CACHE["nc"]
    shared = pack_shared(conv1_w, conv1_b, conv2_w, conv2_b, W)
    in_maps = make_in_maps(x, shared)
    res = run_bass_kernel_spmd(nc, in_maps, list(range(NCORES)), trace=False)
    outs = [res.results[c]["out"] for c in range(NCORES)]
    return np.concatenate(outs, axis=0).reshape(B, C, O).astype(np.float32)
